# revision 24
# baseline (speedup 1.0000x reference)
"""Trainium2 Bass kernel for the alphailp ClauseFunction problem.

Computation (reference):
    idx = I_i[0]                      # [G, S, L] int, values in [0, G)
    gathered = x[:, idx]              # [B, G, S, L]
    body = prod(gathered, axis=-1)    # [B, G, S]
    lse  = gamma * logsumexp(body / gamma, axis=2)   # [B, G]
    m = max(lse); out = lse / m if m > 1 else lse

Device strategy (per NeuronCore, atoms sharded 8 ways):
  - x is transposed on host to a [G, B] table in HBM; every (g, s, l) index
    pulls one contiguous 512B row (all B=128 batch values) via the SWDGE
    dma_gather instruction.  Gather order j = s*128 + a puts atom a on
    partition a and substitution s on the free dim, one gather per literal l.
  - products over L: three in-place DVE tensor_tensor multiplies.
  - softor: body in [0,1) implies body/gamma in [0,100), so a *fixed* shift
    of -64 makes exp(body/gamma - 64) representable in f32 with no per-group
    max pass:  lse = gamma*(64 + ln(sum_s exp(body/gamma - 64))).
  - the global max renormalization needs a cross-core reduction; it is a
    trivial elementwise pass done on host after gathering the 8 shards.
"""

import numpy as np

import concourse.bass as bass
import concourse.tile as tile
from concourse import library_config, mybir
from concourse.bass_utils import run_bass_kernel_spmd
from concourse.vector_clock import ScopedClock

B = 128          # batch
G = 5000         # atoms
S = 32           # substitutions
L = 4            # literals
GAMMA = 0.01
SHIFT = 64.0     # fixed logsumexp shift; exp(body/gamma - SHIFT) never over/underflows
NCORES = 8
GPAD = 5120      # G padded so each core gets 640 atoms = 5 chunks of 128
GC = GPAD // NCORES          # 640 atoms per core
A = 128                      # atoms per chunk (= partition count)
NCHUNK = GC // A             # 5
NIDX = A * S                 # 4096 indices per (chunk, literal) gather
IDXC = NIDX // 16            # 256 idx columns per gather (16-wrapped)

F32 = mybir.dt.float32
I16 = mybir.dt.int16

import os
GBUFS = int(os.environ.get("K_GBUFS", "2"))


def _split_multi_waits(nc: bass.Bass):
    """The pinned walrus accepts at most ONE sem-wait command per TPB
    instruction; Tile emits several.  Move all but one wait of every
    instruction onto freshly inserted same-engine nops placed just before it.

    Engine builders append new instructions to the currently-open block, so
    snapshot every block first, build the new per-block orders, then install
    them (and scrub the builder-appended tail copies from the open block).
    """
    f = nc.m.functions[0]
    cur = nc.cur_bb.bb if hasattr(nc.cur_bb, "bb") else nc.cur_bb
    snaps = {bb.name: list(bb.instructions) for bb in f.blocks}

    def multi(i):
        return i.sync_info and i.sync_info.on_wait and len(i.sync_info.on_wait) > 1

    created = set()
    newlists = {}
    for bb in f.blocks:
        snapshot = snaps[bb.name]
        if not any(multi(i) for i in snapshot):
            continue
        new = []
        for inst in snapshot:
            if multi(inst):
                waits = list(inst.sync_info.on_wait)
                inst.sync_info.on_wait = [waits[-1]]
                for w in waits[:-1]:
                    n = nc.engines[inst.engine].nop(nofuse=True)
                    n.ins.sync_info = mybir.SyncInfo(on_wait=[w], on_update=[])
                    created.add(id(n.ins))
                    new.append(n.ins)
            new.append(inst)
        newlists[bb.name] = new
    for bb in f.blocks:
        if bb.name in newlists:
            bb.instructions = newlists[bb.name]
        elif cur is not None and bb.name == cur.name:
            bb.instructions = [i for i in bb.instructions if id(i) not in created]


def _swap_last_two(ap):
    """View an AP with its last two free dims transposed (stride permutation)."""
    return bass.AP(tensor=ap.tensor, offset=ap.offset,
                   ap=[ap.ap[0], ap.ap[2], ap.ap[1]])


NQUEUES = int(os.environ.get("K_NQUEUES", "4"))


def build_program() -> bass.Bass:
    nc = bass.Bass(num_swdge_queues=NQUEUES)
    xt_ext = nc.declare_dram_parameter("xt", [GPAD, B], F32, isOutput=False)
    idx_ext = nc.declare_dram_parameter(
        "idx", [128, NCHUNK * L * IDXC], I16, isOutput=False)
    out_ext = nc.declare_dram_parameter("out", [128, NCHUNK, B], F32, isOutput=True)

    # dma_gather lives in the 'mlp' GpSimd library overlay; load it up front
    # (outside the TileContext so it precedes every Pool instruction).
    nc.gpsimd.load_library(library_config.mlp)
    # one shared Pool register for every gather's num_idxs (to_reg(int) would
    # otherwise allocate a fresh register per gather and exhaust the file)
    gidx_reg = nc.gpsimd.alloc_register("gidx")
    nc.gpsimd.reg_mov(gidx_reg, NIDX // 4)

    with tile.TileContext(nc) as tc:
        with (
            tc.tile_pool(name="singles", bufs=1) as singles,
            tc.tile_pool(name="gather", bufs=GBUFS) as gpool,
            tc.tile_pool(name="small", bufs=2) as spool,
        ):
            idx_sb = singles.tile([128, NCHUNK * L * IDXC], I16)
            nc.sync.dma_start(out=idx_sb[:], in_=idx_ext[:])
            bias_sb = singles.tile([128, 1], F32, name="bias_sb")
            nc.vector.memset(bias_sb[:], -SHIFT)

            # HW caps one dma_gather at ~1024 indices (descriptor-ring /
            # ucode limit found empirically: 1024 ok, 1152 crashes), so each
            # (chunk, literal) block of 4096 indices is issued as 4 gathers.
            NSPLIT = 4
            GIDX = NIDX // NSPLIT          # 1024 idxs per gather
            GCOL = GIDX // 16              # 64 idx columns per gather
            SSP = S // NSPLIT              # 8 s-slots per gather
            for k in range(NCHUNK):
                Ts = [gpool.tile([128, S, B], F32, name=f"T{l}", tag=f"T{l}")
                      for l in range(L)]
                # round-robin the 4 SWDGE queues so descriptor emission runs
                # on all four Q7 cpu pairs concurrently
                for j in range(NSPLIT):
                    for l in range(L):
                        col = (k * L + l) * IDXC
                        nc.gpsimd.dma_gather(
                            out_ap=Ts[l][:, j * SSP:(j + 1) * SSP, :],
                            in_ap=xt_ext[:],
                            idxs_ap=idx_sb[:, col + j * GCOL:col + (j + 1) * GCOL],
                            num_idxs=GIDX,
                            num_idxs_reg=gidx_reg,
                            elem_size=B,
                            queue_num=l % NQUEUES,
                        )
                # body = T0*T1*T2*T3 (in place into T0 / T2)
                nc.vector.tensor_tensor(
                    out=Ts[0][:], in0=Ts[0][:], in1=Ts[1][:],
                    op=mybir.AluOpType.mult)
                nc.vector.tensor_tensor(
                    out=Ts[2][:], in0=Ts[2][:], in1=Ts[3][:],
                    op=mybir.AluOpType.mult)
                nc.vector.tensor_tensor(
                    out=Ts[0][:], in0=Ts[0][:], in1=Ts[2][:],
                    op=mybir.AluOpType.mult)
                # q = exp(body/gamma - SHIFT) -> T1 (free after first mul)
                nc.scalar.activation(
                    out=Ts[1][:], in_=Ts[0][:],
                    func=mybir.ActivationFunctionType.Exp,
                    bias=bias_sb[:], scale=1.0 / GAMMA)
                # sum over s (innermost via stride-permuted view)
                ssum = spool.tile([128, B], F32, tag="ssum")
                nc.vector.tensor_reduce(
                    out=ssum[:], in_=_swap_last_two(Ts[1][:]),
                    axis=mybir.AxisListType.X, op=mybir.AluOpType.add)
                # lse = gamma*(SHIFT + ln(ssum)).  The HW Ln spline is only
                # accurate on ~[1e-16, 1e14] but ssum spans ~43 decades, so
                # split ssum = mant * 2^(e-127) with DVE bit ops and evaluate
                # Ln only on mant in [1,2):
                #   out = gamma*ln(mant) + gamma*ln2*e + gamma*(SHIFT-127*ln2)
                u_ap = ssum[:].bitcast(mybir.dt.uint32)
                ei = spool.tile([128, B], mybir.dt.uint32, name="ei", tag="ei")
                nc.vector.tensor_scalar(
                    out=ei[:], in0=u_ap, scalar1=23, scalar2=None,
                    op0=mybir.AluOpType.logical_shift_right)
                ef = spool.tile([128, B], F32, name="ef", tag="ef")
                nc.vector.tensor_copy(out=ef[:], in_=ei[:])  # int -> f32
                LN2 = 0.6931471805599453
                nc.vector.tensor_scalar(
                    out=ef[:], in0=ef[:],
                    scalar1=GAMMA * LN2, scalar2=GAMMA * (SHIFT - 127.0 * LN2),
                    op0=mybir.AluOpType.mult, op1=mybir.AluOpType.add)
                mant = spool.tile([128, B], F32, name="mant", tag="mant")
                nc.vector.tensor_scalar(
                    out=mant[:].bitcast(mybir.dt.uint32), in0=u_ap,
                    scalar1=0x007FFFFF, scalar2=0x3F800000,
                    op0=mybir.AluOpType.bitwise_and,
                    op1=mybir.AluOpType.bitwise_or)
                lnm = spool.tile([128, B], F32, name="lnm", tag="lnm")
                nc.scalar.activation(
                    out=lnm[:], in_=mant[:],
                    func=mybir.ActivationFunctionType.Ln)
                ot = spool.tile([128, B], F32, name="ot", tag="ot")
                nc.vector.scalar_tensor_tensor(
                    out=ot[:], in0=lnm[:], scalar=GAMMA, in1=ef[:],
                    op0=mybir.AluOpType.mult, op1=mybir.AluOpType.add)
                nc.sync.dma_start(out=out_ext[:, k, :], in_=ot[:])
    _split_multi_waits(nc)
    # Re-derive each gather's SWDGE queue from the DMASW sem lane Tile
    # assigned (lane % NQUEUES), so every lane is incremented by exactly one
    # queue — required by the per-queue shadow-sem bookkeeping in the ucode.
    for inst in nc.inst_map.values():
        if isinstance(inst, mybir.InstDMAGatherAnt):
            si = inst.sync_info
            if si and si.on_update:
                name = si.on_update[0].ant_name          # e.g. "DMASW5_46"
                lane = int(name.split("_")[0].removeprefix("DMASW"))
                inst.queue_num = lane % NQUEUES
    # populate .instr bytes for bass_isa subclasses (library load, gathers);
    # Bacc.compile does this automatically, plain Bass does not.
    mybir.codegen_inst_isa_subclasses(nc)
    return nc


_CACHED_NC = None


def _get_program():
    global _CACHED_NC
    if _CACHED_NC is None:
        _CACHED_NC = build_program()
    return _CACHED_NC


def _prep_inputs(x: np.ndarray, I_i: np.ndarray):
    x = np.asarray(x, dtype=np.float32)
    idx_full = np.asarray(I_i).astype(np.int64)[0]        # [G, S, L]
    assert x.shape == (B, G) and idx_full.shape == (G, S, L)

    xt = np.zeros((GPAD, B), dtype=np.float32)
    xt[:G] = x.T
    ipad = np.zeros((GPAD, S, L), dtype=np.int16)
    ipad[:G] = idx_full.astype(np.int16)

    idx_maps = []
    for c in range(NCORES):
        blk = ipad[c * GC:(c + 1) * GC]                   # [640, S, L]
        blk = blk.reshape(NCHUNK, A, S, L)                # [k, a, s, l]
        blk = blk.transpose(0, 3, 2, 1)                   # [k, l, s, a]
        blk = blk.reshape(NCHUNK, L, IDXC, 16)            # j = col*16 + p
        blk = blk.transpose(0, 1, 3, 2)                   # [k, l, 16, IDXC]
        blk = np.tile(blk, (1, 1, 8, 1))                  # replicate to 128 parts
        blk = blk.transpose(2, 0, 1, 3).reshape(128, NCHUNK * L * IDXC)
        idx_maps.append(np.ascontiguousarray(blk))
    return xt, idx_maps


def kernel(x: np.ndarray, I_i: np.ndarray) -> np.ndarray:
    nc = _get_program()
    xt, idx_maps = _prep_inputs(x, I_i)
    in_maps = [{"xt": xt, "idx": idx_maps[c]} for c in range(NCORES)]
    res = None
    last_err = None
    for _attempt in range(3):
        try:
            res = run_bass_kernel_spmd(nc, in_maps, list(range(NCORES)))
            break
        except Exception as e:  # transient NRT_EXEC_UNIT_UNRECOVERABLE resets
            last_err = e
            import time as _time
            _time.sleep(2.0)
    if res is None:
        raise last_err
    arr = np.stack([res.results[c]["out"] for c in range(NCORES)])  # [c, a, k, b]
    out = arr.transpose(3, 0, 2, 1).reshape(B, GPAD)[:, :G]
    out = np.ascontiguousarray(out)
    m = out.max()
    if m > 1.0:
        out = out / m
    return out.astype(np.float32)


# revision 25
# speedup vs baseline: 1.0252x; 1.0252x over previous
"""Trainium2 Bass kernel for the alphailp ClauseFunction problem.

Computation (reference):
    idx = I_i[0]                      # [G, S, L] int, values in [0, G)
    gathered = x[:, idx]              # [B, G, S, L]
    body = prod(gathered, axis=-1)    # [B, G, S]
    lse  = gamma * logsumexp(body / gamma, axis=2)   # [B, G]
    m = max(lse); out = lse / m if m > 1 else lse

Device strategy (per NeuronCore, atoms sharded 8 ways):
  - x is transposed on host to a [G, B] table in HBM; every (g, s, l) index
    pulls one contiguous 512B row (all B=128 batch values) via the SWDGE
    dma_gather instruction.  Gather order j = s*128 + a puts atom a on
    partition a and substitution s on the free dim, one gather per literal l.
  - products over L: three in-place DVE tensor_tensor multiplies.
  - softor: body in [0,1) implies body/gamma in [0,100), so a *fixed* shift
    of -64 makes exp(body/gamma - 64) representable in f32 with no per-group
    max pass:  lse = gamma*(64 + ln(sum_s exp(body/gamma - 64))).
  - the global max renormalization needs a cross-core reduction; it is a
    trivial elementwise pass done on host after gathering the 8 shards.
"""

import numpy as np

import concourse.bass as bass
import concourse.tile as tile
from concourse import library_config, mybir
from concourse.bass_utils import run_bass_kernel_spmd
from concourse.vector_clock import ScopedClock

B = 128          # batch
G = 5000         # atoms
S = 32           # substitutions
L = 4            # literals
GAMMA = 0.01
SHIFT = 64.0     # fixed logsumexp shift; exp(body/gamma - SHIFT) never over/underflows
NCORES = 8
GPAD = 5120      # G padded so each core gets 640 atoms = 5 chunks of 128
GC = GPAD // NCORES          # 640 atoms per core
A = 128                      # atoms per chunk (= partition count)
NCHUNK = GC // A             # 5
NIDX = A * S                 # 4096 indices per (chunk, literal) gather
IDXC = NIDX // 16            # 256 idx columns per gather (16-wrapped)

F32 = mybir.dt.float32
I16 = mybir.dt.int16

import os
GBUFS = int(os.environ.get("K_GBUFS", "2"))


def _split_multi_waits(nc: bass.Bass):
    """The pinned walrus accepts at most ONE sem-wait command per TPB
    instruction; Tile emits several.  Move all but one wait of every
    instruction onto freshly inserted same-engine nops placed just before it.

    Engine builders append new instructions to the currently-open block, so
    snapshot every block first, build the new per-block orders, then install
    them (and scrub the builder-appended tail copies from the open block).
    """
    f = nc.m.functions[0]
    cur = nc.cur_bb.bb if hasattr(nc.cur_bb, "bb") else nc.cur_bb
    snaps = {bb.name: list(bb.instructions) for bb in f.blocks}

    def multi(i):
        return i.sync_info and i.sync_info.on_wait and len(i.sync_info.on_wait) > 1

    created = set()
    newlists = {}
    for bb in f.blocks:
        snapshot = snaps[bb.name]
        if not any(multi(i) for i in snapshot):
            continue
        new = []
        for inst in snapshot:
            if multi(inst):
                waits = list(inst.sync_info.on_wait)
                inst.sync_info.on_wait = [waits[-1]]
                for w in waits[:-1]:
                    n = nc.engines[inst.engine].nop(nofuse=True)
                    n.ins.sync_info = mybir.SyncInfo(on_wait=[w], on_update=[])
                    created.add(id(n.ins))
                    new.append(n.ins)
            new.append(inst)
        newlists[bb.name] = new
    for bb in f.blocks:
        if bb.name in newlists:
            bb.instructions = newlists[bb.name]
        elif cur is not None and bb.name == cur.name:
            bb.instructions = [i for i in bb.instructions if id(i) not in created]


def _swap_last_two(ap):
    """View an AP with its last two free dims transposed (stride permutation)."""
    return bass.AP(tensor=ap.tensor, offset=ap.offset,
                   ap=[ap.ap[0], ap.ap[2], ap.ap[1]])


NQUEUES = int(os.environ.get("K_NQUEUES", "4"))


def build_program() -> bass.Bass:
    nc = bass.Bass(num_swdge_queues=NQUEUES)
    xt_ext = nc.declare_dram_parameter("xt", [GPAD, B], F32, isOutput=False)
    idx_ext = nc.declare_dram_parameter(
        "idx", [128, NCHUNK * L * IDXC], I16, isOutput=False)
    out_ext = nc.declare_dram_parameter("out", [128, NCHUNK, B], F32, isOutput=True)

    # dma_gather lives in the 'mlp' GpSimd library overlay; load it up front
    # (outside the TileContext so it precedes every Pool instruction).
    nc.gpsimd.load_library(library_config.mlp)
    # one shared Pool register for every gather's num_idxs (to_reg(int) would
    # otherwise allocate a fresh register per gather and exhaust the file)
    gidx_reg = nc.gpsimd.alloc_register("gidx")
    nc.gpsimd.reg_mov(gidx_reg, NIDX // 4)

    with tile.TileContext(nc) as tc:
        with (
            tc.tile_pool(name="singles", bufs=1) as singles,
            tc.tile_pool(name="gather", bufs=GBUFS) as gpool,
            tc.tile_pool(name="small", bufs=2) as spool,
        ):
            idx_sb = singles.tile([128, NCHUNK * L * IDXC], I16)
            nc.sync.dma_start(out=idx_sb[:], in_=idx_ext[:])
            bias_sb = singles.tile([128, 1], F32, name="bias_sb")
            nc.vector.memset(bias_sb[:], -SHIFT)

            # HW caps one dma_gather at ~1024 indices (descriptor-ring /
            # ucode limit found empirically: 1024 ok, 1152 crashes), so each
            # (chunk, literal) block of 4096 indices is issued as 4 gathers.
            NSPLIT = 4
            GIDX = NIDX // NSPLIT          # 1024 idxs per gather
            GCOL = GIDX // 16              # 64 idx columns per gather
            SSP = S // NSPLIT              # 8 s-slots per gather
            for k in range(NCHUNK):
                Ts = [gpool.tile([128, S, B], F32, name=f"T{l}", tag=f"T{l}")
                      for l in range(L)]
                # round-robin the 4 SWDGE queues so descriptor emission runs
                # on all four Q7 cpu pairs concurrently; products start per
                # j-slice (as soon as its 4 gathers land) so the DVE chain
                # overlaps the chunk's own gather stream instead of trailing it
                for j in range(NSPLIT):
                    for l in range(L):
                        col = (k * L + l) * IDXC
                        nc.gpsimd.dma_gather(
                            out_ap=Ts[l][:, j * SSP:(j + 1) * SSP, :],
                            in_ap=xt_ext[:],
                            idxs_ap=idx_sb[:, col + j * GCOL:col + (j + 1) * GCOL],
                            num_idxs=GIDX,
                            num_idxs_reg=gidx_reg,
                            elem_size=B,
                            queue_num=l % NQUEUES,
                        )
                    sl = (slice(None), slice(j * SSP, (j + 1) * SSP),
                          slice(None))
                    nc.vector.tensor_tensor(
                        out=Ts[0][sl], in0=Ts[0][sl], in1=Ts[1][sl],
                        op=mybir.AluOpType.mult)
                    nc.vector.tensor_tensor(
                        out=Ts[2][sl], in0=Ts[2][sl], in1=Ts[3][sl],
                        op=mybir.AluOpType.mult)
                    nc.vector.tensor_tensor(
                        out=Ts[0][sl], in0=Ts[0][sl], in1=Ts[2][sl],
                        op=mybir.AluOpType.mult)
                # q = exp(body/gamma - SHIFT) -> T1 (free after first mul)
                nc.scalar.activation(
                    out=Ts[1][:], in_=Ts[0][:],
                    func=mybir.ActivationFunctionType.Exp,
                    bias=bias_sb[:], scale=1.0 / GAMMA)
                # sum over s (innermost via stride-permuted view)
                ssum = spool.tile([128, B], F32, tag="ssum")
                nc.vector.tensor_reduce(
                    out=ssum[:], in_=_swap_last_two(Ts[1][:]),
                    axis=mybir.AxisListType.X, op=mybir.AluOpType.add)
                # lse = gamma*(SHIFT + ln(ssum)).  The HW Ln spline is only
                # accurate on ~[1e-16, 1e14] but ssum spans ~43 decades, so
                # split ssum = mant * 2^(e-127) with DVE bit ops and evaluate
                # Ln only on mant in [1,2):
                #   out = gamma*ln(mant) + gamma*ln2*e + gamma*(SHIFT-127*ln2)
                u_ap = ssum[:].bitcast(mybir.dt.uint32)
                ei = spool.tile([128, B], mybir.dt.uint32, name="ei", tag="ei")
                nc.vector.tensor_scalar(
                    out=ei[:], in0=u_ap, scalar1=23, scalar2=None,
                    op0=mybir.AluOpType.logical_shift_right)
                ef = spool.tile([128, B], F32, name="ef", tag="ef")
                nc.vector.tensor_copy(out=ef[:], in_=ei[:])  # int -> f32
                LN2 = 0.6931471805599453
                nc.vector.tensor_scalar(
                    out=ef[:], in0=ef[:],
                    scalar1=GAMMA * LN2, scalar2=GAMMA * (SHIFT - 127.0 * LN2),
                    op0=mybir.AluOpType.mult, op1=mybir.AluOpType.add)
                mant = spool.tile([128, B], F32, name="mant", tag="mant")
                nc.vector.tensor_scalar(
                    out=mant[:].bitcast(mybir.dt.uint32), in0=u_ap,
                    scalar1=0x007FFFFF, scalar2=0x3F800000,
                    op0=mybir.AluOpType.bitwise_and,
                    op1=mybir.AluOpType.bitwise_or)
                lnm = spool.tile([128, B], F32, name="lnm", tag="lnm")
                nc.scalar.activation(
                    out=lnm[:], in_=mant[:],
                    func=mybir.ActivationFunctionType.Ln)
                ot = spool.tile([128, B], F32, name="ot", tag="ot")
                nc.vector.scalar_tensor_tensor(
                    out=ot[:], in0=lnm[:], scalar=GAMMA, in1=ef[:],
                    op0=mybir.AluOpType.mult, op1=mybir.AluOpType.add)
                nc.sync.dma_start(out=out_ext[:, k, :], in_=ot[:])
    _split_multi_waits(nc)
    # Re-derive each gather's SWDGE queue from the DMASW sem lane Tile
    # assigned (lane % NQUEUES), so every lane is incremented by exactly one
    # queue — required by the per-queue shadow-sem bookkeeping in the ucode.
    for inst in nc.inst_map.values():
        if isinstance(inst, mybir.InstDMAGatherAnt):
            si = inst.sync_info
            if si and si.on_update:
                name = si.on_update[0].ant_name          # e.g. "DMASW5_46"
                lane = int(name.split("_")[0].removeprefix("DMASW"))
                inst.queue_num = lane % NQUEUES
    # populate .instr bytes for bass_isa subclasses (library load, gathers);
    # Bacc.compile does this automatically, plain Bass does not.
    mybir.codegen_inst_isa_subclasses(nc)
    return nc


_CACHED_NC = None


def _get_program():
    global _CACHED_NC
    if _CACHED_NC is None:
        _CACHED_NC = build_program()
    return _CACHED_NC


def _prep_inputs(x: np.ndarray, I_i: np.ndarray):
    x = np.asarray(x, dtype=np.float32)
    idx_full = np.asarray(I_i).astype(np.int64)[0]        # [G, S, L]
    assert x.shape == (B, G) and idx_full.shape == (G, S, L)

    xt = np.zeros((GPAD, B), dtype=np.float32)
    xt[:G] = x.T
    ipad = np.zeros((GPAD, S, L), dtype=np.int16)
    ipad[:G] = idx_full.astype(np.int16)

    idx_maps = []
    for c in range(NCORES):
        blk = ipad[c * GC:(c + 1) * GC]                   # [640, S, L]
        blk = blk.reshape(NCHUNK, A, S, L)                # [k, a, s, l]
        blk = blk.transpose(0, 3, 2, 1)                   # [k, l, s, a]
        blk = blk.reshape(NCHUNK, L, IDXC, 16)            # j = col*16 + p
        blk = blk.transpose(0, 1, 3, 2)                   # [k, l, 16, IDXC]
        blk = np.tile(blk, (1, 1, 8, 1))                  # replicate to 128 parts
        blk = blk.transpose(2, 0, 1, 3).reshape(128, NCHUNK * L * IDXC)
        idx_maps.append(np.ascontiguousarray(blk))
    return xt, idx_maps


def kernel(x: np.ndarray, I_i: np.ndarray) -> np.ndarray:
    nc = _get_program()
    xt, idx_maps = _prep_inputs(x, I_i)
    in_maps = [{"xt": xt, "idx": idx_maps[c]} for c in range(NCORES)]
    res = None
    last_err = None
    for _attempt in range(3):
        try:
            res = run_bass_kernel_spmd(nc, in_maps, list(range(NCORES)))
            break
        except Exception as e:  # transient NRT_EXEC_UNIT_UNRECOVERABLE resets
            last_err = e
            import time as _time
            _time.sleep(2.0)
    if res is None:
        raise last_err
    arr = np.stack([res.results[c]["out"] for c in range(NCORES)])  # [c, a, k, b]
    out = arr.transpose(3, 0, 2, 1).reshape(B, GPAD)[:, :G]
    out = np.ascontiguousarray(out)
    m = out.max()
    if m > 1.0:
        out = out / m
    return out.astype(np.float32)


# revision 26
# speedup vs baseline: 1.1083x; 1.0810x over previous
"""Trainium2 Bass kernel for the alphailp ClauseFunction problem.

Computation (reference):
    idx = I_i[0]                      # [G, S, L] int, values in [0, G)
    gathered = x[:, idx]              # [B, G, S, L]
    body = prod(gathered, axis=-1)    # [B, G, S]
    lse  = gamma * logsumexp(body / gamma, axis=2)   # [B, G]
    m = max(lse); out = lse / m if m > 1 else lse

Device strategy (per NeuronCore, atoms sharded 8 ways):
  - x is transposed on host to a [G, B] table in HBM; every (g, s, l) index
    pulls one contiguous 512B row (all B=128 batch values) via the SWDGE
    dma_gather instruction.  Gather order j = s*128 + a puts atom a on
    partition a and substitution s on the free dim, one gather per literal l.
  - products over L: three in-place DVE tensor_tensor multiplies.
  - softor: body in [0,1) implies body/gamma in [0,100), so a *fixed* shift
    of -64 makes exp(body/gamma - 64) representable in f32 with no per-group
    max pass:  lse = gamma*(64 + ln(sum_s exp(body/gamma - 64))).
  - the global max renormalization needs a cross-core reduction; it is a
    trivial elementwise pass done on host after gathering the 8 shards.
"""

import numpy as np

import concourse.bass as bass
import concourse.tile as tile
from concourse import library_config, mybir
from concourse.bass_utils import run_bass_kernel_spmd
from concourse.vector_clock import ScopedClock

B = 128          # batch
G = 5000         # atoms
S = 32           # substitutions
L = 4            # literals
GAMMA = 0.01
SHIFT = 64.0     # fixed logsumexp shift; exp(body/gamma - SHIFT) never over/underflows
NCORES = 8
GPAD = 5120      # G padded so each core gets 640 atoms = 5 chunks of 128
GC = GPAD // NCORES          # 640 atoms per core
A = 128                      # atoms per chunk (= partition count)
NCHUNK = GC // A             # 5
NIDX = A * S                 # 4096 indices per (chunk, literal) gather
IDXC = NIDX // 16            # 256 idx columns per gather (16-wrapped)

F32 = mybir.dt.float32
I16 = mybir.dt.int16

import os
GBUFS = int(os.environ.get("K_GBUFS", "2"))


def _split_multi_waits(nc: bass.Bass):
    """The pinned walrus accepts at most ONE sem-wait command per TPB
    instruction; Tile emits several.  Move all but one wait of every
    instruction onto freshly inserted same-engine nops placed just before it.

    Engine builders append new instructions to the currently-open block, so
    snapshot every block first, build the new per-block orders, then install
    them (and scrub the builder-appended tail copies from the open block).
    """
    f = nc.m.functions[0]
    cur = nc.cur_bb.bb if hasattr(nc.cur_bb, "bb") else nc.cur_bb
    snaps = {bb.name: list(bb.instructions) for bb in f.blocks}

    def multi(i):
        return i.sync_info and i.sync_info.on_wait and len(i.sync_info.on_wait) > 1

    created = set()
    newlists = {}
    for bb in f.blocks:
        snapshot = snaps[bb.name]
        if not any(multi(i) for i in snapshot):
            continue
        new = []
        for inst in snapshot:
            if multi(inst):
                waits = list(inst.sync_info.on_wait)
                inst.sync_info.on_wait = [waits[-1]]
                for w in waits[:-1]:
                    n = nc.engines[inst.engine].nop(nofuse=True)
                    n.ins.sync_info = mybir.SyncInfo(on_wait=[w], on_update=[])
                    created.add(id(n.ins))
                    new.append(n.ins)
            new.append(inst)
        newlists[bb.name] = new
    for bb in f.blocks:
        if bb.name in newlists:
            bb.instructions = newlists[bb.name]
        elif cur is not None and bb.name == cur.name:
            bb.instructions = [i for i in bb.instructions if id(i) not in created]


def _swap_last_two(ap):
    """View an AP with its last two free dims transposed (stride permutation)."""
    return bass.AP(tensor=ap.tensor, offset=ap.offset,
                   ap=[ap.ap[0], ap.ap[2], ap.ap[1]])


NQUEUES = int(os.environ.get("K_NQUEUES", "4"))


def build_program() -> bass.Bass:
    nc = bass.Bass(num_swdge_queues=NQUEUES)
    xt_ext = nc.declare_dram_parameter("xt", [GPAD, B], F32, isOutput=False)
    idx_ext = nc.declare_dram_parameter(
        "idx", [128, NCHUNK * L * IDXC], I16, isOutput=False)
    out_ext = nc.declare_dram_parameter("out", [128, NCHUNK, B], F32, isOutput=True)

    # dma_gather lives in the 'mlp' GpSimd library overlay; load it up front
    # (outside the TileContext so it precedes every Pool instruction).
    nc.gpsimd.load_library(library_config.mlp)
    # one shared Pool register for every gather's num_idxs (to_reg(int) would
    # otherwise allocate a fresh register per gather and exhaust the file)
    gidx_reg = nc.gpsimd.alloc_register("gidx")
    nc.gpsimd.reg_mov(gidx_reg, NIDX // 4)

    with tile.TileContext(nc) as tc:
        with (
            tc.tile_pool(name="singles", bufs=1) as singles,
            tc.tile_pool(name="gather", bufs=GBUFS) as gpool,
            tc.tile_pool(name="small", bufs=2) as spool,
        ):
            idx_sb = singles.tile([128, NCHUNK * L * IDXC], I16)
            nc.sync.dma_start(out=idx_sb[:], in_=idx_ext[:])
            bias_sb = singles.tile([128, 1], F32, name="bias_sb")
            nc.vector.memset(bias_sb[:], -SHIFT)

            # HW caps one dma_gather at ~1024 indices (descriptor-ring /
            # ucode limit found empirically: 1024 ok, 1152 crashes), so each
            # (chunk, literal) block of 4096 indices is issued as 4 gathers.
            NSPLIT = 4
            GIDX = NIDX // NSPLIT          # 1024 idxs per gather
            GCOL = GIDX // 16              # 64 idx columns per gather
            SSP = S // NSPLIT              # 8 s-slots per gather
            for k in range(NCHUNK):
                Ts = [gpool.tile([128, S, B], F32, name=f"T{l}", tag=f"T{l}")
                      for l in range(L)]
                # round-robin the 4 SWDGE queues so descriptor emission runs
                # on all four Q7 cpu pairs concurrently; products start per
                # j-slice (as soon as its 4 gathers land) so the DVE chain
                # overlaps the chunk's own gather stream instead of trailing it
                for j in range(NSPLIT):
                    for l in range(L):
                        col = (k * L + l) * IDXC
                        nc.gpsimd.dma_gather(
                            out_ap=Ts[l][:, j * SSP:(j + 1) * SSP, :],
                            in_ap=xt_ext[:],
                            idxs_ap=idx_sb[:, col + j * GCOL:col + (j + 1) * GCOL],
                            num_idxs=GIDX,
                            num_idxs_reg=gidx_reg,
                            elem_size=B,
                            queue_num=l % NQUEUES,
                        )
                    sl = (slice(None), slice(j * SSP, (j + 1) * SSP),
                          slice(None))
                    nc.vector.tensor_tensor(
                        out=Ts[0][sl], in0=Ts[0][sl], in1=Ts[1][sl],
                        op=mybir.AluOpType.mult)
                    nc.vector.tensor_tensor(
                        out=Ts[2][sl], in0=Ts[2][sl], in1=Ts[3][sl],
                        op=mybir.AluOpType.mult)
                    nc.vector.tensor_tensor(
                        out=Ts[0][sl], in0=Ts[0][sl], in1=Ts[2][sl],
                        op=mybir.AluOpType.mult)
                    # q_j = exp(body_j/gamma - SHIFT) -> T1 slice; partial
                    # s-sum per slice so only ~2us of work trails the last
                    # gather of the chunk
                    nc.scalar.activation(
                        out=Ts[1][sl], in_=Ts[0][sl],
                        func=mybir.ActivationFunctionType.Exp,
                        bias=bias_sb[:], scale=1.0 / GAMMA)
                    psum_j = spool.tile([128, B], F32, name=f"ps{j}",
                                        tag=f"ps{j}")
                    nc.vector.tensor_reduce(
                        out=psum_j[:], in_=_swap_last_two(Ts[1][sl]),
                        axis=mybir.AxisListType.X, op=mybir.AluOpType.add)
                    if j == 0:
                        psums = [psum_j]
                    else:
                        psums.append(psum_j)
                # combine the 4 partial sums
                nc.vector.tensor_tensor(
                    out=psums[0][:], in0=psums[0][:], in1=psums[1][:],
                    op=mybir.AluOpType.add)
                nc.vector.tensor_tensor(
                    out=psums[2][:], in0=psums[2][:], in1=psums[3][:],
                    op=mybir.AluOpType.add)
                ssum = spool.tile([128, B], F32, tag="ssum")
                nc.vector.tensor_tensor(
                    out=ssum[:], in0=psums[0][:], in1=psums[2][:],
                    op=mybir.AluOpType.add)
                # lse = gamma*(SHIFT + ln(ssum)).  The HW Ln spline is only
                # accurate on ~[1e-16, 1e14] but ssum spans ~43 decades, so
                # split ssum = mant * 2^(e-127) with DVE bit ops and evaluate
                # Ln only on mant in [1,2):
                #   out = gamma*ln(mant) + gamma*ln2*e + gamma*(SHIFT-127*ln2)
                u_ap = ssum[:].bitcast(mybir.dt.uint32)
                ei = spool.tile([128, B], mybir.dt.uint32, name="ei", tag="ei")
                nc.vector.tensor_scalar(
                    out=ei[:], in0=u_ap, scalar1=23, scalar2=None,
                    op0=mybir.AluOpType.logical_shift_right)
                ef = spool.tile([128, B], F32, name="ef", tag="ef")
                nc.vector.tensor_copy(out=ef[:], in_=ei[:])  # int -> f32
                LN2 = 0.6931471805599453
                nc.vector.tensor_scalar(
                    out=ef[:], in0=ef[:],
                    scalar1=GAMMA * LN2, scalar2=GAMMA * (SHIFT - 127.0 * LN2),
                    op0=mybir.AluOpType.mult, op1=mybir.AluOpType.add)
                mant = spool.tile([128, B], F32, name="mant", tag="mant")
                nc.vector.tensor_scalar(
                    out=mant[:].bitcast(mybir.dt.uint32), in0=u_ap,
                    scalar1=0x007FFFFF, scalar2=0x3F800000,
                    op0=mybir.AluOpType.bitwise_and,
                    op1=mybir.AluOpType.bitwise_or)
                lnm = spool.tile([128, B], F32, name="lnm", tag="lnm")
                nc.scalar.activation(
                    out=lnm[:], in_=mant[:],
                    func=mybir.ActivationFunctionType.Ln)
                ot = spool.tile([128, B], F32, name="ot", tag="ot")
                nc.vector.scalar_tensor_tensor(
                    out=ot[:], in0=lnm[:], scalar=GAMMA, in1=ef[:],
                    op0=mybir.AluOpType.mult, op1=mybir.AluOpType.add)
                nc.sync.dma_start(out=out_ext[:, k, :], in_=ot[:])
    _split_multi_waits(nc)
    # Re-derive each gather's SWDGE queue from the DMASW sem lane Tile
    # assigned (lane % NQUEUES), so every lane is incremented by exactly one
    # queue — required by the per-queue shadow-sem bookkeeping in the ucode.
    for inst in nc.inst_map.values():
        if isinstance(inst, mybir.InstDMAGatherAnt):
            si = inst.sync_info
            if si and si.on_update:
                name = si.on_update[0].ant_name          # e.g. "DMASW5_46"
                lane = int(name.split("_")[0].removeprefix("DMASW"))
                inst.queue_num = lane % NQUEUES
    # populate .instr bytes for bass_isa subclasses (library load, gathers);
    # Bacc.compile does this automatically, plain Bass does not.
    mybir.codegen_inst_isa_subclasses(nc)
    return nc


_CACHED_NC = None


def _get_program():
    global _CACHED_NC
    if _CACHED_NC is None:
        _CACHED_NC = build_program()
    return _CACHED_NC


def _prep_inputs(x: np.ndarray, I_i: np.ndarray):
    x = np.asarray(x, dtype=np.float32)
    idx_full = np.asarray(I_i).astype(np.int64)[0]        # [G, S, L]
    assert x.shape == (B, G) and idx_full.shape == (G, S, L)

    xt = np.zeros((GPAD, B), dtype=np.float32)
    xt[:G] = x.T
    ipad = np.zeros((GPAD, S, L), dtype=np.int16)
    ipad[:G] = idx_full.astype(np.int16)

    idx_maps = []
    for c in range(NCORES):
        blk = ipad[c * GC:(c + 1) * GC]                   # [640, S, L]
        blk = blk.reshape(NCHUNK, A, S, L)                # [k, a, s, l]
        blk = blk.transpose(0, 3, 2, 1)                   # [k, l, s, a]
        blk = blk.reshape(NCHUNK, L, IDXC, 16)            # j = col*16 + p
        blk = blk.transpose(0, 1, 3, 2)                   # [k, l, 16, IDXC]
        blk = np.tile(blk, (1, 1, 8, 1))                  # replicate to 128 parts
        blk = blk.transpose(2, 0, 1, 3).reshape(128, NCHUNK * L * IDXC)
        idx_maps.append(np.ascontiguousarray(blk))
    return xt, idx_maps


def kernel(x: np.ndarray, I_i: np.ndarray) -> np.ndarray:
    nc = _get_program()
    xt, idx_maps = _prep_inputs(x, I_i)
    in_maps = [{"xt": xt, "idx": idx_maps[c]} for c in range(NCORES)]
    res = None
    last_err = None
    for _attempt in range(3):
        try:
            res = run_bass_kernel_spmd(nc, in_maps, list(range(NCORES)))
            break
        except Exception as e:  # transient NRT_EXEC_UNIT_UNRECOVERABLE resets
            last_err = e
            import time as _time
            _time.sleep(2.0)
    if res is None:
        raise last_err
    arr = np.stack([res.results[c]["out"] for c in range(NCORES)])  # [c, a, k, b]
    out = arr.transpose(3, 0, 2, 1).reshape(B, GPAD)[:, :G]
    out = np.ascontiguousarray(out)
    m = out.max()
    if m > 1.0:
        out = out / m
    return out.astype(np.float32)


# revision 27
# speedup vs baseline: 1.1287x; 1.0184x over previous
"""Trainium2 Bass kernel for the alphailp ClauseFunction problem.

Computation (reference):
    idx = I_i[0]                      # [G, S, L] int, values in [0, G)
    gathered = x[:, idx]              # [B, G, S, L]
    body = prod(gathered, axis=-1)    # [B, G, S]
    lse  = gamma * logsumexp(body / gamma, axis=2)   # [B, G]
    m = max(lse); out = lse / m if m > 1 else lse

Device strategy (per NeuronCore, atoms sharded 8 ways):
  - x is transposed on host to a [G, B] table in HBM; every (g, s, l) index
    pulls one contiguous 512B row (all B=128 batch values) via the SWDGE
    dma_gather instruction.  Gather order j = s*128 + a puts atom a on
    partition a and substitution s on the free dim, one gather per literal l.
  - products over L: three in-place DVE tensor_tensor multiplies.
  - softor: body in [0,1) implies body/gamma in [0,100), so a *fixed* shift
    of -64 makes exp(body/gamma - 64) representable in f32 with no per-group
    max pass:  lse = gamma*(64 + ln(sum_s exp(body/gamma - 64))).
  - the global max renormalization needs a cross-core reduction; it is a
    trivial elementwise pass done on host after gathering the 8 shards.
"""

import numpy as np

import concourse.bass as bass
import concourse.tile as tile
from concourse import library_config, mybir
from concourse.bass_utils import run_bass_kernel_spmd
from concourse.vector_clock import ScopedClock

B = 128          # batch
G = 5000         # atoms
S = 32           # substitutions
L = 4            # literals
GAMMA = 0.01
SHIFT = 64.0     # fixed logsumexp shift; exp(body/gamma - SHIFT) never over/underflows
NCORES = 8
GPAD = 5120      # G padded so each core gets 640 atoms = 5 chunks of 128
GC = GPAD // NCORES          # 640 atoms per core
A = 128                      # atoms per chunk (= partition count)
NCHUNK = GC // A             # 5
NIDX = A * S                 # 4096 indices per (chunk, literal) gather
IDXC = NIDX // 16            # 256 idx columns per gather (16-wrapped)

F32 = mybir.dt.float32
I16 = mybir.dt.int16

import os
GBUFS = int(os.environ.get("K_GBUFS", "2"))


def _split_multi_waits(nc: bass.Bass):
    """The pinned walrus accepts at most ONE sem-wait command per TPB
    instruction; Tile emits several.  Move all but one wait of every
    instruction onto freshly inserted same-engine nops placed just before it.

    Engine builders append new instructions to the currently-open block, so
    snapshot every block first, build the new per-block orders, then install
    them (and scrub the builder-appended tail copies from the open block).
    """
    f = nc.m.functions[0]
    cur = nc.cur_bb.bb if hasattr(nc.cur_bb, "bb") else nc.cur_bb
    snaps = {bb.name: list(bb.instructions) for bb in f.blocks}

    def multi(i):
        return i.sync_info and i.sync_info.on_wait and len(i.sync_info.on_wait) > 1

    created = set()
    newlists = {}
    for bb in f.blocks:
        snapshot = snaps[bb.name]
        if not any(multi(i) for i in snapshot):
            continue
        new = []
        for inst in snapshot:
            if multi(inst):
                waits = list(inst.sync_info.on_wait)
                inst.sync_info.on_wait = [waits[-1]]
                for w in waits[:-1]:
                    n = nc.engines[inst.engine].nop(nofuse=True)
                    n.ins.sync_info = mybir.SyncInfo(on_wait=[w], on_update=[])
                    created.add(id(n.ins))
                    new.append(n.ins)
            new.append(inst)
        newlists[bb.name] = new
    for bb in f.blocks:
        if bb.name in newlists:
            bb.instructions = newlists[bb.name]
        elif cur is not None and bb.name == cur.name:
            bb.instructions = [i for i in bb.instructions if id(i) not in created]


def _swap_last_two(ap):
    """View an AP with its last two free dims transposed (stride permutation)."""
    return bass.AP(tensor=ap.tensor, offset=ap.offset,
                   ap=[ap.ap[0], ap.ap[2], ap.ap[1]])


NQUEUES = int(os.environ.get("K_NQUEUES", "4"))


def build_program() -> bass.Bass:
    nc = bass.Bass(num_swdge_queues=NQUEUES)
    xt_ext = nc.declare_dram_parameter("xt", [GPAD, B], F32, isOutput=False)
    idx_ext = nc.declare_dram_parameter(
        "idx", [128, NCHUNK * L * IDXC], I16, isOutput=False)
    out_ext = nc.declare_dram_parameter("out", [128, NCHUNK, B], F32, isOutput=True)

    # dma_gather lives in the 'mlp' GpSimd library overlay; load it up front
    # (outside the TileContext so it precedes every Pool instruction).
    nc.gpsimd.load_library(library_config.mlp)
    # one shared Pool register for every gather's num_idxs (to_reg(int) would
    # otherwise allocate a fresh register per gather and exhaust the file)
    gidx_reg = nc.gpsimd.alloc_register("gidx")
    nc.gpsimd.reg_mov(gidx_reg, NIDX // 4)

    with tile.TileContext(nc) as tc:
        with (
            tc.tile_pool(name="singles", bufs=1) as singles,
            tc.tile_pool(name="gather", bufs=GBUFS) as gpool,
            tc.tile_pool(name="small", bufs=2) as spool,
        ):
            idx_sb = singles.tile([128, NCHUNK * L * IDXC], I16)
            # per-chunk idx loads so chunk 0's gathers only wait on its own
            # slice (the monolithic 1.25MB load gated the whole stream)
            for kk in range(NCHUNK):
                c0 = kk * L * IDXC
                nc.sync.dma_start(out=idx_sb[:, c0:c0 + L * IDXC],
                                  in_=idx_ext[:, c0:c0 + L * IDXC])
            bias_sb = singles.tile([128, 1], F32, name="bias_sb")
            nc.vector.memset(bias_sb[:], -SHIFT)

            # HW caps one dma_gather at ~1024 indices (descriptor-ring /
            # ucode limit found empirically: 1024 ok, 1152 crashes), so each
            # (chunk, literal) block of 4096 indices is issued as 4 gathers.
            NSPLIT = 4
            GIDX = NIDX // NSPLIT          # 1024 idxs per gather
            GCOL = GIDX // 16              # 64 idx columns per gather
            SSP = S // NSPLIT              # 8 s-slots per gather
            for k in range(NCHUNK):
                Ts = [gpool.tile([128, S, B], F32, name=f"T{l}", tag=f"T{l}")
                      for l in range(L)]
                # round-robin the 4 SWDGE queues so descriptor emission runs
                # on all four Q7 cpu pairs concurrently; products start per
                # j-slice (as soon as its 4 gathers land) so the DVE chain
                # overlaps the chunk's own gather stream instead of trailing it
                for j in range(NSPLIT):
                    for l in range(L):
                        col = (k * L + l) * IDXC
                        nc.gpsimd.dma_gather(
                            out_ap=Ts[l][:, j * SSP:(j + 1) * SSP, :],
                            in_ap=xt_ext[:],
                            idxs_ap=idx_sb[:, col + j * GCOL:col + (j + 1) * GCOL],
                            num_idxs=GIDX,
                            num_idxs_reg=gidx_reg,
                            elem_size=B,
                            queue_num=l % NQUEUES,
                        )
                    sl = (slice(None), slice(j * SSP, (j + 1) * SSP),
                          slice(None))
                    nc.vector.tensor_tensor(
                        out=Ts[0][sl], in0=Ts[0][sl], in1=Ts[1][sl],
                        op=mybir.AluOpType.mult)
                    nc.vector.tensor_tensor(
                        out=Ts[2][sl], in0=Ts[2][sl], in1=Ts[3][sl],
                        op=mybir.AluOpType.mult)
                    nc.vector.tensor_tensor(
                        out=Ts[0][sl], in0=Ts[0][sl], in1=Ts[2][sl],
                        op=mybir.AluOpType.mult)
                    # q_j = exp(body_j/gamma - SHIFT) -> T1 slice; partial
                    # s-sum per slice so only ~2us of work trails the last
                    # gather of the chunk
                    nc.scalar.activation(
                        out=Ts[1][sl], in_=Ts[0][sl],
                        func=mybir.ActivationFunctionType.Exp,
                        bias=bias_sb[:], scale=1.0 / GAMMA)
                    psum_j = spool.tile([128, B], F32, name=f"ps{j}",
                                        tag=f"ps{j}")
                    nc.vector.tensor_reduce(
                        out=psum_j[:], in_=_swap_last_two(Ts[1][sl]),
                        axis=mybir.AxisListType.X, op=mybir.AluOpType.add)
                    if j == 0:
                        psums = [psum_j]
                    else:
                        psums.append(psum_j)
                # combine the 4 partial sums
                nc.vector.tensor_tensor(
                    out=psums[0][:], in0=psums[0][:], in1=psums[1][:],
                    op=mybir.AluOpType.add)
                nc.vector.tensor_tensor(
                    out=psums[2][:], in0=psums[2][:], in1=psums[3][:],
                    op=mybir.AluOpType.add)
                ssum = spool.tile([128, B], F32, tag="ssum")
                nc.vector.tensor_tensor(
                    out=ssum[:], in0=psums[0][:], in1=psums[2][:],
                    op=mybir.AluOpType.add)
                # lse = gamma*(SHIFT + ln(ssum)).  The HW Ln spline is only
                # accurate on ~[1e-16, 1e14] but ssum spans ~43 decades, so
                # split ssum = mant * 2^(e-127) with DVE bit ops and evaluate
                # Ln only on mant in [1,2):
                #   out = gamma*ln(mant) + gamma*ln2*e + gamma*(SHIFT-127*ln2)
                u_ap = ssum[:].bitcast(mybir.dt.uint32)
                ei = spool.tile([128, B], mybir.dt.uint32, name="ei", tag="ei")
                nc.vector.tensor_scalar(
                    out=ei[:], in0=u_ap, scalar1=23, scalar2=None,
                    op0=mybir.AluOpType.logical_shift_right)
                ef = spool.tile([128, B], F32, name="ef", tag="ef")
                nc.vector.tensor_copy(out=ef[:], in_=ei[:])  # int -> f32
                LN2 = 0.6931471805599453
                nc.vector.tensor_scalar(
                    out=ef[:], in0=ef[:],
                    scalar1=GAMMA * LN2, scalar2=GAMMA * (SHIFT - 127.0 * LN2),
                    op0=mybir.AluOpType.mult, op1=mybir.AluOpType.add)
                mant = spool.tile([128, B], F32, name="mant", tag="mant")
                nc.vector.tensor_scalar(
                    out=mant[:].bitcast(mybir.dt.uint32), in0=u_ap,
                    scalar1=0x007FFFFF, scalar2=0x3F800000,
                    op0=mybir.AluOpType.bitwise_and,
                    op1=mybir.AluOpType.bitwise_or)
                lnm = spool.tile([128, B], F32, name="lnm", tag="lnm")
                nc.scalar.activation(
                    out=lnm[:], in_=mant[:],
                    func=mybir.ActivationFunctionType.Ln)
                ot = spool.tile([128, B], F32, name="ot", tag="ot")
                nc.vector.scalar_tensor_tensor(
                    out=ot[:], in0=lnm[:], scalar=GAMMA, in1=ef[:],
                    op0=mybir.AluOpType.mult, op1=mybir.AluOpType.add)
                nc.sync.dma_start(out=out_ext[:, k, :], in_=ot[:])
    _split_multi_waits(nc)
    # Re-derive each gather's SWDGE queue from the DMASW sem lane Tile
    # assigned (lane % NQUEUES), so every lane is incremented by exactly one
    # queue — required by the per-queue shadow-sem bookkeeping in the ucode.
    for inst in nc.inst_map.values():
        if isinstance(inst, mybir.InstDMAGatherAnt):
            si = inst.sync_info
            if si and si.on_update:
                name = si.on_update[0].ant_name          # e.g. "DMASW5_46"
                lane = int(name.split("_")[0].removeprefix("DMASW"))
                inst.queue_num = lane % NQUEUES
    # populate .instr bytes for bass_isa subclasses (library load, gathers);
    # Bacc.compile does this automatically, plain Bass does not.
    mybir.codegen_inst_isa_subclasses(nc)
    return nc


_CACHED_NC = None


def _get_program():
    global _CACHED_NC
    if _CACHED_NC is None:
        _CACHED_NC = build_program()
    return _CACHED_NC


def _prep_inputs(x: np.ndarray, I_i: np.ndarray):
    x = np.asarray(x, dtype=np.float32)
    idx_full = np.asarray(I_i).astype(np.int64)[0]        # [G, S, L]
    assert x.shape == (B, G) and idx_full.shape == (G, S, L)

    xt = np.zeros((GPAD, B), dtype=np.float32)
    xt[:G] = x.T
    ipad = np.zeros((GPAD, S, L), dtype=np.int16)
    ipad[:G] = idx_full.astype(np.int16)

    idx_maps = []
    for c in range(NCORES):
        blk = ipad[c * GC:(c + 1) * GC]                   # [640, S, L]
        blk = blk.reshape(NCHUNK, A, S, L)                # [k, a, s, l]
        blk = blk.transpose(0, 3, 2, 1)                   # [k, l, s, a]
        blk = blk.reshape(NCHUNK, L, IDXC, 16)            # j = col*16 + p
        blk = blk.transpose(0, 1, 3, 2)                   # [k, l, 16, IDXC]
        blk = np.tile(blk, (1, 1, 8, 1))                  # replicate to 128 parts
        blk = blk.transpose(2, 0, 1, 3).reshape(128, NCHUNK * L * IDXC)
        idx_maps.append(np.ascontiguousarray(blk))
    return xt, idx_maps


def kernel(x: np.ndarray, I_i: np.ndarray) -> np.ndarray:
    nc = _get_program()
    xt, idx_maps = _prep_inputs(x, I_i)
    in_maps = [{"xt": xt, "idx": idx_maps[c]} for c in range(NCORES)]
    res = None
    last_err = None
    for _attempt in range(3):
        try:
            res = run_bass_kernel_spmd(nc, in_maps, list(range(NCORES)))
            break
        except Exception as e:  # transient NRT_EXEC_UNIT_UNRECOVERABLE resets
            last_err = e
            import time as _time
            _time.sleep(2.0)
    if res is None:
        raise last_err
    arr = np.stack([res.results[c]["out"] for c in range(NCORES)])  # [c, a, k, b]
    out = arr.transpose(3, 0, 2, 1).reshape(B, GPAD)[:, :G]
    out = np.ascontiguousarray(out)
    m = out.max()
    if m > 1.0:
        out = out / m
    return out.astype(np.float32)
